# revision 12
# baseline (speedup 1.0000x reference)
"""Location-dependent 3D conv (AsymConv) on 8 TRN2 NeuronCores.

Math (per output voxel):
    out[b, 0, x, y, z] = sum_{i,j,l in 0..2} Xp[b, x+i, y+j, z+l] * W[x, y, z, (i*3+j)*3+l]
with Xp = edge-padded X by 1 plane on each spatial side.

Strategy (v2, rebuilt from trace analysis of the 60 us baseline):
  - Shard the X spatial axis (96 = 8 cores x 12 planes); host ships halo'd
    shards -> no inter-core communication.
  - Per-core SBUF layout: partition dim = y (96 of 128); free = (b, x, z).
    y-shifts come from 3 host-shipped pre-shifted copies (compute APs must
    start at partition 0/32/64/96); x/z shifts are free-dim offsets.
  - Products X*W run on DVE in fp16 2x mode as fused TRIPLE ops: taps grouped
    by (i, l) with the 3 j-taps expressed as one extra AP dim over the packed
    x/W tiles -> 18 big ops instead of 54, minimal per-op overhead.
  - A few triples are offloaded to the otherwise-idle GPSIMD engine
    (tensor_tensor never enters the DVE 2-port mode, so no port contention).
  - The 27-term accumulation runs on the TensorEngine as identity matmuls
    into PSUM fp32 (chunks 512/512/128 columns). Dummy warm-up matmuls during
    the DMA phase ramp the PE clock 1.2 -> 2.4 GHz before real work arrives.
  - DMA: few BIG HWDGE transfers (W as per-triple 663 kB slices of one packed
    tile, 96 descriptors x 6.9 kB each) split across the SP and ACT rings,
    ordered by consumption; compute starts ~2.5 us in (baseline: 11.5 us).
  - l==1 taps need 4B-aligned reads for DVE 2x mode -> ScalarE builds
    z-shifted copies of the X tiles while DMAs stream.
  - PSUM -> SBUF fp16 (ScalarE + DVE) -> DRAM; host upcasts and reassembles.
"""

import os

import numpy as np

# ---- problem constants (hardcoded per harness rules) ----
B = 2
D = 96  # Dx = Dy = Dz
KSZ = 3
NTAP = KSZ**3  # 27
NCORES = 8
XS = D // NCORES  # 12 x-planes per core
XH = XS + 2  # with halo
ZP = D + 2  # padded z
FREE = XS * D  # 1152 free elements per (tap, batch) stream
NTRI = 9  # tap triples (i, l), each covering j = 0..2

F16 = np.float16
LAST_RESULT = None  # BassKernelResults of the most recent run (for test.py)

_GRAPH_CACHE = {}

# triple order of consumption: l=0 first, then l=2, then l=1 (z-shifted
# copies needed by l==1 are built by ScalarE while the early taps stream)
TRI_ORDER = [(0, 0), (1, 0), (2, 0), (0, 2), (1, 2), (2, 2), (0, 1), (1, 1), (2, 1)]

# PE warm-up matmuls during the DMA phase (HAM clock ramp 1.2 -> 2.4 GHz)
N_WARMUP = int(os.environ.get("ASYM_WARMUP", "10"))
# triples computed on GPSIMD as (tri_idx, batch) pairs; DVE does the rest.
# default: T2 both batches + T5 b0  (~22.5 us GPSIMD, ~27.5 us DVE)
GPS_PLAN = os.environ.get("ASYM_GPS_PLAN", "2:0,2:1,5:0")
# psum x-plane chunks of the 12-plane streams (480/480/192 columns)
CHUNKS = [(0, 5), (5, 5), (10, 2)]


def _parse_gps_plan():
    plan = set()
    if GPS_PLAN.strip():
        for item in GPS_PLAN.split(","):
            t, b = item.split(":")
            plan.add((int(t), int(b)))
    return plan


def _build_graph():
    """Build (and cache) the per-core Bass graph. Same graph for all 8 cores."""
    if "nc" in _GRAPH_CACHE:
        return _GRAPH_CACHE["nc"]

    from concourse import bacc
    from concourse import bass as _bass
    import concourse.mybir as mybir
    from concourse.tile import TileContext

    f16 = mybir.dt.float16
    f32 = mybir.dt.float32

    gps_plan = _parse_gps_plan()

    nc = bacc.Bacc("TRN2", target_bir_lowering=False, debug=False, num_devices=NCORES)

    # y-pre-shifted X copies: xj[y', b, x, z] = Xp[y'+j, b, x, z]
    x_ds = [
        nc.dram_tensor(f"x{j}", [D, B, XH, ZP], f16, kind="ExternalInput")
        for j in range(KSZ)
    ]
    # W packed per triple (i, l) in TRI_ORDER, y-major to match the SBUF tile
    w_d = nc.dram_tensor("w", [D, NTRI, KSZ, XS, D], f16, kind="ExternalInput")
    id_d = nc.dram_tensor("ident", [D, D], f16, kind="ExternalInput")
    out_d = nc.dram_tensor("out", [D, B, XS, D], f16, kind="ExternalOutput")

    with TileContext(nc) as tc:
        with (
            tc.tile_pool(name="xp", bufs=1) as xpool,
            tc.tile_pool(name="wp", bufs=1) as wpool,
            tc.tile_pool(name="pp", bufs=6) as ppool,
            tc.tile_pool(name="psp", bufs=1, space="PSUM") as pspool,
        ):
            # ---- static tiles ----
            x_all = xpool.tile([D, KSZ, B, XH, ZP], f16, name="x_all", tag="x_all")
            # z-shifted (by 1) copy for l==1 taps; x-pitch padded to ZP so all
            # AP base offsets stay 4B-aligned (2x DVE mode)
            x_z1 = xpool.tile([D, KSZ, B, XH, ZP], f16, name="x_z1", tag="x_z1")
            w_all = wpool.tile(
                [D, NTRI, KSZ, XS, D], f16, name="w_all", tag="w_all"
            )
            id_t = xpool.tile([D, D], f16, name="id_t", tag="id_t")

            # ---- DMA schedule ----
            # ACT ring: ident + first W triple (j-split for earliest start) + T1
            nc.scalar.dma_start(out=id_t[:], in_=id_d.ap())
            nc.scalar.dma_start(out=w_all[:, 0:1, 0:1], in_=w_d.ap()[:, 0:1, 0:1])
            nc.scalar.dma_start(out=w_all[:, 0:1, 1:3], in_=w_d.ap()[:, 0:1, 1:3])
            nc.scalar.dma_start(out=w_all[:, 1:2], in_=w_d.ap()[:, 1:2])
            # SP ring: X slices (b-split for earliest first product), then the
            # W triple groups in consumption order
            for j in range(KSZ):
                for b in range(B):
                    nc.sync.dma_start(
                        out=x_all[:, j : j + 1, b : b + 1],
                        in_=x_ds[j].ap()[:, b : b + 1],
                    )
            nc.sync.dma_start(out=w_all[:, 2:4], in_=w_d.ap()[:, 2:4])
            nc.sync.dma_start(out=w_all[:, 4:6], in_=w_d.ap()[:, 4:6])
            nc.sync.dma_start(out=w_all[:, 6:8], in_=w_d.ap()[:, 6:8])
            nc.sync.dma_start(out=w_all[:, 8:9], in_=w_d.ap()[:, 8:9])

            # ---- PE warm-up during the DMA phase ----
            if N_WARMUP:
                dummy = ppool.tile([D, 480], f16, name="dummy", tag="warm_rhs", bufs=1)
                nc.vector.memset(dummy[:], 0.0)
                ps_w = pspool.tile([D, 480], f32, name="ps_warm", tag="ps_warm")
                for _ in range(N_WARMUP):
                    nc.tensor.matmul(ps_w[:], id_t[:], dummy[:], start=True, stop=True)

            # ---- ScalarE: z-shifted copies (for l == 1 triples) ----
            for j in range(KSZ):
                nc.scalar.copy(
                    out=x_z1[:, j : j + 1, :, :, 0 : ZP - 1],
                    in_=x_all[:, j : j + 1, :, :, 1:ZP],
                )

            # ---- product + accumulate schedule ----
            psums = {
                (b, ci): pspool.tile(
                    [D, nx, D], f32, name=f"ps_{b}_{ci}", tag=f"ps_{b}_{ci}"
                )
                for b in range(B)
                for ci, (x0, nx) in enumerate(CHUNKS)
            }

            def x_src_ap(ti, i, l, b, j0, nj):
                """[D, nj, XS, D] view of x_all/x_z1 for triple (i, l), batch b."""
                base_t = x_z1 if l == 1 else x_all
                ll = 0 if l == 1 else l
                v = base_t[:, j0 : j0 + nj, b, i : i + XS, ll : ll + D]
                return v

            # per (b, chunk) accumulation counters for start/stop flags
            nstreams = NTRI * KSZ
            seen = {(b, ci): 0 for b in range(B) for ci in range(len(CHUNKS))}

            def consume(prod, b, nj):
                """PE: accumulate nj streams of a product tile into psums[b]."""
                for ci, (x0, nx) in enumerate(CHUNKS):
                    for j in range(nj):
                        s = seen[(b, ci)]
                        nc.tensor.matmul(
                            psums[(b, ci)][:],
                            id_t[:],
                            prod[:, j, x0 : x0 + nx, :],
                            start=(s == 0),
                            stop=(s == nstreams - 1),
                        )
                        seen[(b, ci)] = s + 1

            # T0 as per-j singles (X/W land j-incrementally), rest as triples
            for ti, (i, l) in enumerate(TRI_ORDER):
                if ti == 0:
                    for j in range(KSZ):
                        for b in range(B):
                            prod = ppool.tile(
                                [D, 1, XS, D], f16, name="prod1", tag="prod1", bufs=4
                            )
                            nc.vector.tensor_mul(
                                out=prod[:],
                                in0=x_src_ap(ti, i, l, b, j, 1),
                                in1=w_all[:, ti, j : j + 1, :, :],
                            )
                            consume(prod, b, 1)
                    continue
                for b in range(B):
                    eng = nc.gpsimd if (ti, b) in gps_plan else nc.vector
                    prod = ppool.tile(
                        [D, KSZ, XS, D], f16, name="prod3", tag="prod3", bufs=5
                    )
                    eng.tensor_mul(
                        out=prod[:],
                        in0=x_src_ap(ti, i, l, b, 0, KSZ),
                        in1=w_all[:, ti, :, :, :],
                    )
                    consume(prod, b, KSZ)

            # ---- evacuate PSUM -> SBUF f16 -> DRAM ----
            for b in range(B):
                for ci, (x0, nx) in enumerate(CHUNKS):
                    outsb = ppool.tile(
                        [D, nx, D], f16, name="outsb", tag=f"outsb_{b}_{ci}", bufs=1
                    )
                    if b == 0:
                        nc.scalar.copy(out=outsb[:], in_=psums[(b, ci)][:])
                    else:
                        nc.vector.tensor_copy(out=outsb[:], in_=psums[(b, ci)][:])
                    q = nc.sync if ci % 2 == 0 else nc.scalar
                    q.dma_start(
                        out=out_d.ap()[:, b, x0 : x0 + nx, :],
                        in_=outsb[:],
                    )

    nc.compile()
    _GRAPH_CACHE["nc"] = nc
    return nc


def make_in_maps(X, W):
    """Host-side shard prep. X [2,1,96,96,96] f32, W [1,1,96,96,96,27] f32."""
    X = np.asarray(X)
    W = np.asarray(W)
    Xs = X.reshape(B, D, D, D)
    # edge padding on all three spatial dims
    Xp = np.pad(Xs, ((0, 0), (1, 1), (1, 1), (1, 1)), mode="edge")
    # -> [y, b, x, z]
    Xt = np.ascontiguousarray(np.transpose(Xp, (2, 0, 1, 3))).astype(F16)
    W00 = W.reshape(D, D, D, NTAP)  # [x, y, z, tap]
    ident = np.eye(D, dtype=F16)

    in_maps = []
    for m in range(NCORES):
        xs_full = Xt[:, :, m * XS : m * XS + XH, :]  # [98, 2, 14, 98]
        im = {"ident": ident}
        for j in range(KSZ):
            im[f"x{j}"] = np.ascontiguousarray(xs_full[j : j + D])
        # W triple layout [y, tri, j, x, z] (y-major, matches the SBUF tile)
        wm = W00[m * XS : (m + 1) * XS]  # [12, 96, 96, 27]
        wt = np.empty((D, NTRI, KSZ, XS, D), dtype=F16)
        for ti, (i, l) in enumerate(TRI_ORDER):
            for j in range(KSZ):
                t = (i * KSZ + j) * KSZ + l
                wt[:, ti, j] = np.transpose(wm[:, :, :, t], (1, 0, 2))
        im["w"] = np.ascontiguousarray(wt)
        in_maps.append(im)
    return in_maps


def kernel(X, W):
    global LAST_RESULT
    from concourse.bass_utils import run_bass_kernel_spmd

    nc = _build_graph()
    in_maps = make_in_maps(X, W)
    trace = bool(int(os.environ.get("ASYM_TRACE", "0")))
    res = run_bass_kernel_spmd(
        nc, in_maps, core_ids=list(range(NCORES)), trace=trace
    )
    LAST_RESULT = res

    out = np.empty((B, 1, D, D, D), dtype=np.float32)
    for m in range(NCORES):
        r = res.results[m]["out"].astype(np.float32)  # [y, b, x, z]
        out[:, 0, m * XS : (m + 1) * XS, :, :] = np.transpose(r, (1, 2, 0, 3))
    return out


# revision 13
# speedup vs baseline: 1.2994x; 1.2994x over previous
"""Location-dependent 3D conv (AsymConv) on 8 TRN2 NeuronCores.

Math (per output voxel):
    out[b, 0, x, y, z] = sum_{i,j,l in 0..2} Xp[b, x+i, y+j, z+l] * W[x, y, z, (i*3+j)*3+l]
with Xp = edge-padded X by 1 plane on each spatial side.

Strategy (v2, rebuilt from trace analysis of the 60 us baseline):
  - Shard the X spatial axis (96 = 8 cores x 12 planes); host ships halo'd
    shards -> no inter-core communication.
  - Per-core SBUF layout: partition dim = y (96 of 128); free = (b, x, z).
    y-shifts come from 3 host-shipped pre-shifted copies (compute APs must
    start at partition 0/32/64/96); x/z shifts are free-dim offsets.
  - Products X*W run on the DVE in fp16 2x mode (~0.6 us per tap-batch is the
    hard DVE floor; GPSIMD "help" was measured to contend and lose). Taps are
    fused pairwise via small-stride overlapping access patterns: (l=0, l=2)
    z-pairs and, for l==1, (i=0, i=2) x-pairs - 24 pair + 6 single ops.
  - The 27-term accumulation runs on the TensorEngine as identity matmuls
    into PSUM fp32 (x-chunks 5/5/2 planes). Dummy warm-up matmuls during the
    DMA phase ramp the PE clock 1.2 -> 2.4 GHz before real work arrives.
  - DMA: few BIG HWDGE transfers in consumption order on both rings (W pair
    tiles are host-packed so each is ONE contiguous 442 kB transfer with
    4.6 kB descriptors); separate tiles per unit avoid false write-after-read
    serialization of the W stream. Compute starts ~2.5 us in (baseline: 11.5).
  - l==1 taps need 4B-aligned reads for DVE 2x mode -> ScalarE builds
    z-shifted X copies during the load phase.
  - PSUM -> SBUF fp16 (ScalarE for b0, DVE cast for b1) -> DRAM; host upcasts
    and reassembles.
"""

import os

import numpy as np

# ---- problem constants (hardcoded per harness rules) ----
B = 2
D = 96  # Dx = Dy = Dz
KSZ = 3
NTAP = KSZ**3  # 27
NCORES = 8
XS = D // NCORES  # 12 x-planes per core
XH = XS + 2  # with halo
ZP = D + 2  # padded z

F16 = np.float16
LAST_RESULT = None  # BassKernelResults of the most recent run (for test.py)

_GRAPH_CACHE = {}

N_WARMUP = int(os.environ.get("ASYM_WARMUP", "10"))
# psum x-plane chunks of the 12-plane streams (480/480/192 columns)
CHUNKS = [(0, 5), (5, 5), (10, 2)]

# ---- product unit schedule ----
# kinds: "s0" = first-unit singles (taps (0,0,0) and (0,0,2), issued as two
# singles so the very first op only waits for half of the first W tile),
# "p" = (l0, l2) z-pair for (i, j), "q" = (i0, i2) x-pair for l=1, j,
# "s1" = single (1, j, 1).
# consumption order pairs with the j-availability of the X tiles.
UNITS = [
    ("s0", (0, 0)),
    ("p", (1, 0)),
    ("p", (2, 0)),
    ("p", (0, 1)),
    ("p", (1, 1)),
    ("p", (2, 1)),
    ("p", (0, 2)),
    ("p", (1, 2)),
    ("p", (2, 2)),
    ("q", 0),
    ("q", 1),
    ("q", 2),
    ("s1", 0),
    ("s1", 1),
    ("s1", 2),
]


def _unit_taps(kind, arg):
    if kind == "s0":
        i, j = arg
        return [(i, j, 0), (i, j, 2)]
    if kind == "p":
        i, j = arg
        return [(i, j, 0), (i, j, 2)]
    if kind == "q":
        j = arg
        return [(0, j, 1), (2, j, 1)]
    j = arg
    return [(1, j, 1)]


def _build_graph():
    """Build (and cache) the per-core Bass graph. Same graph for all 8 cores."""
    if "nc" in _GRAPH_CACHE:
        return _GRAPH_CACHE["nc"]

    from concourse import bacc
    from concourse import bass as _bass
    import concourse.mybir as mybir
    from concourse.tile import TileContext

    f16 = mybir.dt.float16
    f32 = mybir.dt.float32

    nc = bacc.Bacc("TRN2", target_bir_lowering=False, debug=False, num_devices=NCORES)

    # y-pre-shifted X copies: xj[y', b, x, z] = Xp[y'+j, b, x, z]
    x_ds = [
        nc.dram_tensor(f"x{j}", [D, B, XH, ZP], f16, kind="ExternalInput")
        for j in range(KSZ)
    ]
    # W per unit, y-major: pairs [D, 2, XS, D], singles [D, XS, D]
    w_ds = []
    for ui, (kind, arg) in enumerate(UNITS):
        nt = len(_unit_taps(kind, arg))
        shape = [D, nt, XS, D] if nt == 2 else [D, XS, D]
        w_ds.append(nc.dram_tensor(f"w{ui}", shape, f16, kind="ExternalInput"))
    id_d = nc.dram_tensor("ident", [D, D], f16, kind="ExternalInput")
    out_d = nc.dram_tensor("out", [D, B, XS, D], f16, kind="ExternalOutput")

    with TileContext(nc) as tc:
        with (
            tc.tile_pool(name="xp", bufs=1) as xpool,
            tc.tile_pool(name="wp", bufs=1) as wpool,
            tc.tile_pool(name="pp", bufs=6) as ppool,
            tc.tile_pool(name="psp", bufs=1, space="PSUM") as pspool,
        ):
            # ---- static tiles ----
            x_ts = [
                xpool.tile([D, B, XH, ZP], f16, name=f"x_{j}", tag=f"x_{j}")
                for j in range(KSZ)
            ]
            # z-shifted copies for l == 1 (keeps DVE 2x alignment)
            xz_ts = [
                xpool.tile([D, B, XH, ZP], f16, name=f"xz_{j}", tag=f"xz_{j}")
                for j in range(KSZ)
            ]
            w_ts = []
            for ui, (kind, arg) in enumerate(UNITS):
                nt = len(_unit_taps(kind, arg))
                shape = [D, nt, XS, D] if nt == 2 else [D, XS, D]
                w_ts.append(wpool.tile(shape, f16, name=f"w_{ui}", tag=f"w_{ui}"))
            id_t = xpool.tile([D, D], f16, name="id_t", tag="id_t")

            # ---- DMA schedule (HWDGE, both rings, consumption order) ----
            # ACT ring: ident + first W tiles
            nc.scalar.dma_start(out=id_t[:], in_=id_d.ap())
            nc.scalar.dma_start(out=w_ts[0][:, 0:1], in_=w_ds[0].ap()[:, 0:1])
            nc.scalar.dma_start(out=w_ts[0][:, 1:2], in_=w_ds[0].ap()[:, 1:2])
            nc.scalar.dma_start(out=w_ts[1][:], in_=w_ds[1].ap())
            # SP ring: X slices (b-split for earliest first product) + rest of
            # the W stream, interleaved in consumption order
            nc.sync.dma_start(out=x_ts[0][:, 0:1], in_=x_ds[0].ap()[:, 0:1])
            nc.sync.dma_start(out=x_ts[0][:, 1:2], in_=x_ds[0].ap()[:, 1:2])
            nc.sync.dma_start(out=w_ts[2][:], in_=w_ds[2].ap())
            nc.sync.dma_start(out=x_ts[1][:, 0:1], in_=x_ds[1].ap()[:, 0:1])
            nc.sync.dma_start(out=x_ts[1][:, 1:2], in_=x_ds[1].ap()[:, 1:2])
            nc.sync.dma_start(out=w_ts[3][:], in_=w_ds[3].ap())
            nc.sync.dma_start(out=x_ts[2][:, 0:1], in_=x_ds[2].ap()[:, 0:1])
            nc.sync.dma_start(out=x_ts[2][:, 1:2], in_=x_ds[2].ap()[:, 1:2])
            for ui in range(4, 9):
                nc.sync.dma_start(out=w_ts[ui][:], in_=w_ds[ui].ap())
            # ACT ring: l=1 W tiles (consumed last)
            for ui in range(9, len(UNITS)):
                nc.scalar.dma_start(out=w_ts[ui][:], in_=w_ds[ui].ap())

            # ---- PE warm-up during the DMA phase ----
            if N_WARMUP:
                dummy = ppool.tile([D, 480], f16, name="dummy", tag="warm_rhs", bufs=1)
                nc.vector.memset(dummy[:], 0.0)
                ps_w = pspool.tile([D, 480], f32, name="ps_warm", tag="ps_warm")
                for _ in range(N_WARMUP):
                    nc.tensor.matmul(ps_w[:], id_t[:], dummy[:], start=True, stop=True)

            # ---- ScalarE: z-shifted copies (for l == 1 units) ----
            for j in range(KSZ):
                nc.scalar.copy(
                    out=xz_ts[j][:, :, :, 0 : ZP - 1], in_=x_ts[j][:, :, :, 1:ZP]
                )

            # ---- product + accumulate schedule ----
            psums = {
                (b, ci): pspool.tile(
                    [D, nx, D], f32, name=f"ps_{b}_{ci}", tag=f"ps_{b}_{ci}"
                )
                for b in range(B)
                for ci, (x0, nx) in enumerate(CHUNKS)
            }

            def zpair_ap(j, b, i):
                """[D, 2, XS, D] view of x_ts[j]: overlapping z-windows l=0,2."""
                base = x_ts[j][:, b, i : i + XS, 0:D]
                ap = list(base.ap)
                return _bass.AP(
                    base.tensor, base.offset, [ap[0], [2, 2], ap[1], ap[2]]
                )

            def xpair_ap(j, b):
                """[D, 2, XS, D] view of xz_ts[j]: overlapping x-windows i=0,2."""
                base = xz_ts[j][:, b, 0:XS, 0:D]
                ap = list(base.ap)
                return _bass.AP(
                    base.tensor, base.offset, [ap[0], [2 * ap[1][0], 2], ap[1], ap[2]]
                )

            # per (b, chunk) accumulation counters for start/stop flags
            seen = {(b, ci): 0 for b in range(B) for ci in range(len(CHUNKS))}

            def consume(prod, b, nt):
                """PE: accumulate nt tap-streams of a product tile into psums."""
                for ci, (x0, nx) in enumerate(CHUNKS):
                    for t in range(nt):
                        s = seen[(b, ci)]
                        rhs = (
                            prod[:, t, x0 : x0 + nx, :]
                            if nt == 2
                            else prod[:, x0 : x0 + nx, :]
                        )
                        nc.tensor.matmul(
                            psums[(b, ci)][:],
                            id_t[:],
                            rhs,
                            start=(s == 0),
                            stop=(s == NTAP - 1),
                        )
                        seen[(b, ci)] = s + 1

            for ui, (kind, arg) in enumerate(UNITS):
                for b in range(B):
                    if kind == "s0":
                        # two singles: first waits only on half the W tile
                        i, j = arg
                        for s, l in enumerate((0, 2)):
                            prod = ppool.tile(
                                [D, XS, D], f16, name="prod1", tag="prod1", bufs=4
                            )
                            nc.vector.tensor_mul(
                                out=prod[:],
                                in0=x_ts[j][:, b, i : i + XS, l : l + D],
                                in1=w_ts[ui][:, s],
                            )
                            consume(prod, b, 1)
                        continue
                    if kind == "p":
                        i, j = arg
                        src = zpair_ap(j, b, i)
                    elif kind == "q":
                        j = arg
                        src = xpair_ap(j, b)
                    else:
                        j = arg
                        src = xz_ts[j][:, b, 1 : 1 + XS, 0:D]
                    nt = len(_unit_taps(kind, arg))
                    if nt == 2:
                        prod = ppool.tile(
                            [D, 2, XS, D], f16, name="prod2", tag="prod2", bufs=5
                        )
                    else:
                        prod = ppool.tile(
                            [D, XS, D], f16, name="prod1", tag="prod1", bufs=4
                        )
                    nc.vector.tensor_mul(out=prod[:], in0=src, in1=w_ts[ui][:])
                    consume(prod, b, nt)

            # ---- evacuate PSUM -> SBUF f16 -> DRAM ----
            for b in range(B):
                for ci, (x0, nx) in enumerate(CHUNKS):
                    outsb = ppool.tile(
                        [D, nx, D], f16, name="outsb", tag=f"outsb_{b}_{ci}", bufs=1
                    )
                    if b == 0:
                        nc.scalar.copy(out=outsb[:], in_=psums[(b, ci)][:])
                    else:
                        nc.vector.tensor_copy(out=outsb[:], in_=psums[(b, ci)][:])
                    q = nc.sync if ci % 2 == 0 else nc.scalar
                    q.dma_start(
                        out=out_d.ap()[:, b, x0 : x0 + nx, :],
                        in_=outsb[:],
                    )

    nc.compile()
    _GRAPH_CACHE["nc"] = nc
    return nc


def make_in_maps(X, W):
    """Host-side shard prep. X [2,1,96,96,96] f32, W [1,1,96,96,96,27] f32."""
    X = np.asarray(X)
    W = np.asarray(W)
    Xs = X.reshape(B, D, D, D)
    # edge padding on all three spatial dims
    Xp = np.pad(Xs, ((0, 0), (1, 1), (1, 1), (1, 1)), mode="edge")
    # -> [y, b, x, z]
    Xt = np.ascontiguousarray(np.transpose(Xp, (2, 0, 1, 3))).astype(F16)
    W00 = W.reshape(D, D, D, NTAP)  # [x, y, z, tap]
    ident = np.eye(D, dtype=F16)

    in_maps = []
    for m in range(NCORES):
        xs_full = Xt[:, :, m * XS : m * XS + XH, :]  # [98, 2, 14, 98]
        im = {"ident": ident}
        for j in range(KSZ):
            im[f"x{j}"] = np.ascontiguousarray(xs_full[j : j + D])
        wm = W00[m * XS : (m + 1) * XS]  # [12, 96, 96, 27]
        wmt = np.transpose(wm, (1, 0, 2, 3))  # [y, x, z, tap]
        for ui, (kind, arg) in enumerate(UNITS):
            taps = _unit_taps(kind, arg)
            idxs = [(i * KSZ + j) * KSZ + l for (i, j, l) in taps]
            blk = wmt[:, :, :, idxs]  # [y, x, z, nt]
            if len(idxs) == 2:
                wt = np.transpose(blk, (0, 3, 1, 2))  # [y, nt, x, z]
            else:
                wt = blk[:, :, :, 0]  # [y, x, z]
            im[f"w{ui}"] = np.ascontiguousarray(wt).astype(F16)
        in_maps.append(im)
    return in_maps


def kernel(X, W):
    global LAST_RESULT
    from concourse.bass_utils import run_bass_kernel_spmd

    nc = _build_graph()
    in_maps = make_in_maps(X, W)
    trace = bool(int(os.environ.get("ASYM_TRACE", "0")))
    res = run_bass_kernel_spmd(
        nc, in_maps, core_ids=list(range(NCORES)), trace=trace
    )
    LAST_RESULT = res

    out = np.empty((B, 1, D, D, D), dtype=np.float32)
    for m in range(NCORES):
        r = res.results[m]["out"].astype(np.float32)  # [y, b, x, z]
        out[:, 0, m * XS : (m + 1) * XS, :, :] = np.transpose(r, (1, 2, 0, 3))
    return out


# revision 18
# speedup vs baseline: 1.4406x; 1.1086x over previous
"""Location-dependent 3D conv (AsymConv) on 8 TRN2 NeuronCores.

Math (per output voxel):
    out[b, 0, x, y, z] = sum_{i,j,l in 0..2} Xp[b, x+i, y+j, z+l] * W[x, y, z, (i*3+j)*3+l]
with Xp = edge-padded X by 1 plane on each spatial side.

Strategy (v2, rebuilt from trace analysis of the 60 us baseline):
  - Shard the X spatial axis (96 = 8 cores x 12 planes); host ships halo'd
    shards -> no inter-core communication.
  - Per-core SBUF layout: partition dim = y (96 of 128); free = (b, x, z).
    y-shifts come from 3 host-shipped pre-shifted copies (compute APs must
    start at partition 0/32/64/96); x/z shifts are free-dim offsets.
  - Products X*W run on the DVE in fp16 2x mode (~0.6 us per tap-batch is the
    hard DVE floor; GPSIMD "help" was measured to contend and lose). Taps are
    fused pairwise via small-stride overlapping access patterns: (l=0, l=2)
    z-pairs and, for l==1, (i=0, i=2) x-pairs - 24 pair + 6 single ops.
  - The 27-term accumulation runs on the TensorEngine as identity matmuls
    into PSUM fp32 (x-chunks 5/5/2 planes). Dummy warm-up matmuls during the
    DMA phase ramp the PE clock 1.2 -> 2.4 GHz before real work arrives.
  - DMA: few BIG HWDGE transfers in consumption order on both rings (W pair
    tiles are host-packed so each is ONE contiguous 442 kB transfer with
    4.6 kB descriptors); separate tiles per unit avoid false write-after-read
    serialization of the W stream. Compute starts ~2.5 us in (baseline: 11.5).
  - l==1 taps need 4B-aligned reads for DVE 2x mode -> ScalarE builds
    z-shifted X copies during the load phase.
  - PSUM -> SBUF fp16 (ScalarE for b0, DVE cast for b1) -> DRAM; host upcasts
    and reassembles.
"""

import os

import numpy as np

# ---- problem constants (hardcoded per harness rules) ----
B = 2
D = 96  # Dx = Dy = Dz
KSZ = 3
NTAP = KSZ**3  # 27
NCORES = 8
XS = D // NCORES  # 12 x-planes per core
XH = XS + 2  # with halo
ZP = D + 2  # padded z

F16 = np.float16
LAST_RESULT = None  # BassKernelResults of the most recent run (for test.py)

_GRAPH_CACHE = {}

N_WARMUP = int(os.environ.get("ASYM_WARMUP", "10"))
# psum x-plane chunks of the 12-plane streams (480/480/192 columns)
CHUNKS = [(0, 5), (5, 5), (10, 2)]

# ---- product unit schedule ----
# kinds: "s0" = first-unit singles (taps (0,0,0) and (0,0,2), issued as two
# singles so the very first op only waits for half of the first W tile),
# "p" = (l0, l2) z-pair for (i, j), "q" = (i0, i2) x-pair for l=1, j,
# "s1" = single (1, j, 1).
# consumption order pairs with the j-availability of the X tiles.
UNITS = [
    ("s0", (0, 0)),
    ("p", (1, 0)),
    ("p", (2, 0)),
    ("p", (0, 1)),
    ("p", (1, 1)),
    ("p", (2, 1)),
    ("p", (0, 2)),
    ("p", (1, 2)),
    ("p", (2, 2)),
    ("q", 0),
    ("q", 1),
    ("q", 2),
    ("s1", 0),
    ("s1", 1),
    ("s1", 2),
]


def _unit_taps(kind, arg):
    if kind == "s0":
        i, j = arg
        return [(i, j, 0), (i, j, 2)]
    if kind == "p":
        i, j = arg
        return [(i, j, 0), (i, j, 2)]
    if kind == "q":
        j = arg
        return [(0, j, 1), (2, j, 1)]
    j = arg
    return [(1, j, 1)]


def _build_graph():
    """Build (and cache) the per-core Bass graph. Same graph for all 8 cores."""
    if "nc" in _GRAPH_CACHE:
        return _GRAPH_CACHE["nc"]

    from concourse import bacc
    from concourse import bass as _bass
    import concourse.mybir as mybir
    from concourse.tile import TileContext

    f16 = mybir.dt.float16
    f32 = mybir.dt.float32

    nc = bacc.Bacc("TRN2", target_bir_lowering=False, debug=False, num_devices=NCORES)

    # y-pre-shifted X copies packed on one tensor: x[y', j, b, x, z]
    x_d = nc.dram_tensor("x", [D, KSZ, B, XH, ZP], f16, kind="ExternalInput")
    # W per unit (y-major), all units concatenated along the free dim:
    # pair units are 2*XS*D = 2304 elems wide, singles XS*D = 1152
    unit_w = [len(_unit_taps(k, a)) * XS * D for (k, a) in UNITS]
    unit_off = np.concatenate([[0], np.cumsum(unit_w)]).tolist()
    w_d = nc.dram_tensor("w", [D, unit_off[-1]], f16, kind="ExternalInput")
    id_d = nc.dram_tensor("ident", [D, D], f16, kind="ExternalInput")
    out_d = nc.dram_tensor("out", [D, B, XS, D], f16, kind="ExternalOutput")

    with TileContext(nc) as tc:
        with (
            tc.tile_pool(name="xp", bufs=1) as xpool,
            tc.tile_pool(name="wp", bufs=1) as wpool,
            tc.tile_pool(name="pp", bufs=6) as ppool,
            tc.tile_pool(name="psp", bufs=1, space="PSUM") as pspool,
        ):
            # ---- static tiles ----
            x_ts = [
                xpool.tile([D, B, XH, ZP], f16, name=f"x_{j}", tag=f"x_{j}")
                for j in range(KSZ)
            ]
            # z-shifted copies for l == 1 (keeps DVE 2x alignment)
            xz_ts = [
                xpool.tile([D, B, XH, ZP], f16, name=f"xz_{j}", tag=f"xz_{j}")
                for j in range(KSZ)
            ]
            w_ts = []
            for ui, (kind, arg) in enumerate(UNITS):
                nt = len(_unit_taps(kind, arg))
                shape = [D, nt, XS, D] if nt == 2 else [D, XS, D]
                w_ts.append(wpool.tile(shape, f16, name=f"w_{ui}", tag=f"w_{ui}"))
            id_t = xpool.tile([D, D], f16, name="id_t", tag="id_t")

            # ---- DMA schedule (HWDGE, both rings, consumption order) ----
            def w_dma(q, ui, half=None):
                o0, o1 = unit_off[ui], unit_off[ui + 1]
                if half is None:
                    q.dma_start(out=w_ts[ui][:], in_=w_d.ap()[:, o0:o1])
                else:
                    h = (o1 - o0) // 2
                    q.dma_start(
                        out=w_ts[ui][:, half : half + 1],
                        in_=w_d.ap()[:, o0 + half * h : o0 + (half + 1) * h],
                    )

            # ACT ring: ident + first W tiles
            nc.scalar.dma_start(out=id_t[:], in_=id_d.ap())
            w_dma(nc.scalar, 0, half=0)
            w_dma(nc.scalar, 0, half=1)
            w_dma(nc.scalar, 1)
            # SP ring: X slices (b-split for earliest first product) + rest of
            # the W stream, interleaved in consumption order
            nc.sync.dma_start(out=x_ts[0][:, 0:1], in_=x_d.ap()[:, 0, 0:1])
            nc.sync.dma_start(out=x_ts[0][:, 1:2], in_=x_d.ap()[:, 0, 1:2])
            w_dma(nc.sync, 2)
            nc.sync.dma_start(out=x_ts[1][:, 0:1], in_=x_d.ap()[:, 1, 0:1])
            nc.sync.dma_start(out=x_ts[1][:, 1:2], in_=x_d.ap()[:, 1, 1:2])
            w_dma(nc.sync, 3)
            nc.sync.dma_start(out=x_ts[2][:, 0:1], in_=x_d.ap()[:, 2, 0:1])
            nc.sync.dma_start(out=x_ts[2][:, 1:2], in_=x_d.ap()[:, 2, 1:2])
            for ui in range(4, 9):
                w_dma(nc.sync, ui)

            # ---- PE warm-up during the DMA phase (no DMA dependency: uses
            # the memset dummy as both weights and rhs) ----
            if N_WARMUP:
                dummy = ppool.tile([D, 480], f16, name="dummy", tag="warm_rhs", bufs=1)
                nc.vector.memset(dummy[:], 0.0)
                ps_w = pspool.tile([D, 480], f32, name="ps_warm", tag="ps_warm")
                for _ in range(N_WARMUP):
                    nc.tensor.matmul(
                        ps_w[:], dummy[:, 0:D], dummy[:], start=True, stop=True
                    )

            # ---- ScalarE: z-shifted copies (for l == 1 units), interleaved
            # with the late l=1 W dispatches so neither blocks the other ----
            for j in range(KSZ):
                nc.scalar.copy(
                    out=xz_ts[j][:, :, :, 0 : ZP - 1], in_=x_ts[j][:, :, :, 1:ZP]
                )
                w_dma(nc.scalar, 9 + j)
            for ui in range(12, len(UNITS)):
                w_dma(nc.scalar, ui)

            # ---- product + accumulate schedule ----
            psums = {
                (b, ci): pspool.tile(
                    [D, nx, D], f32, name=f"ps_{b}_{ci}", tag=f"ps_{b}_{ci}"
                )
                for b in range(B)
                for ci, (x0, nx) in enumerate(CHUNKS)
            }

            def zpair_ap(j, b, i):
                """[D, 2, XS, D] view of x_ts[j]: overlapping z-windows l=0,2."""
                base = x_ts[j][:, b, i : i + XS, 0:D]
                ap = list(base.ap)
                return _bass.AP(
                    base.tensor, base.offset, [ap[0], [2, 2], ap[1], ap[2]]
                )

            def xpair_ap(j, b):
                """[D, 2, XS, D] view of xz_ts[j]: overlapping x-windows i=0,2."""
                base = xz_ts[j][:, b, 0:XS, 0:D]
                ap = list(base.ap)
                return _bass.AP(
                    base.tensor, base.offset, [ap[0], [2 * ap[1][0], 2], ap[1], ap[2]]
                )

            # per (b, chunk) accumulation counters for start/stop flags
            seen = {(b, ci): 0 for b in range(B) for ci in range(len(CHUNKS))}

            def consume(prod, b, nt):
                """PE: accumulate nt tap-streams of a product tile into psums.
                Chunk-inner order: consecutive matmuls hit different PSUM banks
                (same-bank back-to-back stalls the accumulate pipeline)."""
                for t in range(nt):
                    for ci, (x0, nx) in enumerate(CHUNKS):
                        s = seen[(b, ci)]
                        rhs = (
                            prod[:, t, x0 : x0 + nx, :]
                            if nt == 2
                            else prod[:, x0 : x0 + nx, :]
                        )
                        nc.tensor.matmul(
                            psums[(b, ci)][:],
                            id_t[:],
                            rhs,
                            start=(s == 0),
                            stop=(s == NTAP - 1),
                        )
                        seen[(b, ci)] = s + 1

            for ui, (kind, arg) in enumerate(UNITS):
                for b in range(B):
                    if kind == "s0":
                        # two singles: first waits only on half the W tile
                        i, j = arg
                        for s, l in enumerate((0, 2)):
                            prod = ppool.tile(
                                [D, XS, D], f16, name="prod1", tag="prod1", bufs=4
                            )
                            nc.vector.tensor_mul(
                                out=prod[:],
                                in0=x_ts[j][:, b, i : i + XS, l : l + D],
                                in1=w_ts[ui][:, s],
                            )
                            consume(prod, b, 1)
                        continue
                    if kind == "p":
                        i, j = arg
                        src = zpair_ap(j, b, i)
                    elif kind == "q":
                        j = arg
                        src = xpair_ap(j, b)
                    else:
                        j = arg
                        src = xz_ts[j][:, b, 1 : 1 + XS, 0:D]
                    nt = len(_unit_taps(kind, arg))
                    if nt == 2:
                        prod = ppool.tile(
                            [D, 2, XS, D], f16, name="prod2", tag="prod2", bufs=5
                        )
                    else:
                        prod = ppool.tile(
                            [D, XS, D], f16, name="prod1", tag="prod1", bufs=4
                        )
                    nc.vector.tensor_mul(out=prod[:], in0=src, in1=w_ts[ui][:])
                    consume(prod, b, nt)

            # ---- evacuate PSUM -> SBUF f16 -> DRAM ----
            for b in range(B):
                for ci, (x0, nx) in enumerate(CHUNKS):
                    outsb = ppool.tile(
                        [D, nx, D], f16, name="outsb", tag=f"outsb_{b}_{ci}", bufs=1
                    )
                    if b == 0:
                        nc.scalar.copy(out=outsb[:], in_=psums[(b, ci)][:])
                    else:
                        nc.vector.tensor_copy(out=outsb[:], in_=psums[(b, ci)][:])
                    q = nc.sync if ci % 2 == 0 else nc.scalar
                    q.dma_start(
                        out=out_d.ap()[:, b, x0 : x0 + nx, :],
                        in_=outsb[:],
                    )

    nc.compile()
    _GRAPH_CACHE["nc"] = nc
    return nc


def make_in_maps(X, W):
    """Host-side shard prep. X [2,1,96,96,96] f32, W [1,1,96,96,96,27] f32."""
    X = np.asarray(X)
    W = np.asarray(W)
    Xs = X.reshape(B, D, D, D)
    # edge padding on all three spatial dims
    Xp = np.pad(Xs, ((0, 0), (1, 1), (1, 1), (1, 1)), mode="edge")
    # -> [y, b, x, z]
    Xt = np.ascontiguousarray(np.transpose(Xp, (2, 0, 1, 3))).astype(F16)
    W00 = W.reshape(D, D, D, NTAP)  # [x, y, z, tap]
    ident = np.eye(D, dtype=F16)

    in_maps = []
    for m in range(NCORES):
        xs_full = Xt[:, :, m * XS : m * XS + XH, :]  # [98, 2, 14, 98]
        im = {"ident": ident}
        im["x"] = np.ascontiguousarray(
            np.stack([xs_full[j : j + D] for j in range(KSZ)], axis=1)
        )  # [96, 3, 2, 14, 98]
        wm = W00[m * XS : (m + 1) * XS]  # [12, 96, 96, 27]
        wmt = np.transpose(wm, (1, 0, 2, 3))  # [y, x, z, tap]
        blocks = []
        for kind, arg in UNITS:
            taps = _unit_taps(kind, arg)
            idxs = [(i * KSZ + j) * KSZ + l for (i, j, l) in taps]
            blk = wmt[:, :, :, idxs]  # [y, x, z, nt]
            if len(idxs) == 2:
                wt = np.transpose(blk, (0, 3, 1, 2))  # [y, nt, x, z]
            else:
                wt = blk[:, :, :, 0]  # [y, x, z]
            blocks.append(wt.reshape(D, -1))
        im["w"] = np.ascontiguousarray(np.concatenate(blocks, axis=1)).astype(F16)
        in_maps.append(im)
    return in_maps


def kernel(X, W):
    global LAST_RESULT
    from concourse.bass_utils import run_bass_kernel_spmd

    nc = _build_graph()
    in_maps = make_in_maps(X, W)
    trace = bool(int(os.environ.get("ASYM_TRACE", "0")))
    res = run_bass_kernel_spmd(
        nc, in_maps, core_ids=list(range(NCORES)), trace=trace
    )
    LAST_RESULT = res

    out = np.empty((B, 1, D, D, D), dtype=np.float32)
    for m in range(NCORES):
        r = res.results[m]["out"].astype(np.float32)  # [y, b, x, z]
        out[:, 0, m * XS : (m + 1) * XS, :, :] = np.transpose(r, (1, 2, 0, 3))
    return out
